# revision 1
# baseline (speedup 1.0000x reference)
"""Trainium2 Bass kernel for nn_DirectionalContrastiveLoss (8-core SPMD).

Strategy (per spec sharding hint): shard the anchor/row dimension across the 8
cores, replicate the host-assembled memory bank, compute each core's score
block locally, and combine masked sums / counts on the host.

Layout tricks:
- Rows are sorted by memory_labels with fixed per-label quotas so every core
  gets an identical label layout -> one SPMD program for all 8 cores.
- Bank columns are sorted by the (transposed-bug) anchor-label vector of each
  direction, so the label mask becomes per-row contiguous column ranges
  ("killed" ranges) handled by a few extra ACT accumulate instructions.
- Matmul runs in bf16 (features pre-scaled by 1/TEMP) with fp32 PSUM
  accumulation; softmax statistics use a per-PSUM-fill flash combine.
"""
import numpy as np
import ml_dtypes

import bass_rust
import concourse.bass as bass
import concourse.tile as tile
from concourse import mybir
from concourse.bass_utils import run_bass_kernel_spmd
from concourse.vector_clock import ScopedClock

BF16 = ml_dtypes.bfloat16
N_CORES = 8
TEMP = 0.1
POS_THRESH = 0.7
EPS = 1e-8
N = 8000          # anchors (== memory slots)
C = 256           # feature channels
NLAB = 21         # pseudo-label values 0..20
RPC = 1024        # rows per core per direction (padded)
NT = RPC // 128   # row tiles per direction
FILL_MAX = 1024   # PSUM fill width (2 banks of fp32)
MM_CHUNK = 512    # matmul free-dim chunk (1 PSUM bank)
RMAX = 6          # max label runs per 128-row tile

LAST_RESULTS = None  # BassKernelResults of the most recent kernel() call

# ---------------------------------------------------------------------------
# walrus in this toolchain rejects >1 sync wait per instruction; spread the
# TileContext tail-drain waits over single-wait sync NOPs.
_N_SPILL_NOPS = 64


def _patched_drain_and_barrier(self, tick_clock, wait_clock):
    nops = [self.nc.sync.nop(nofuse=True, hint=f"drainwait{i}")
            for i in range(_N_SPILL_NOPS)]
    drain_inst = self.nc.sync.drain()
    wait_clock.add_sem_waits(drain_inst.ins,
                             ScopedClock({None: tick_clock.global_clock}))
    si = drain_inst.ins.sync_info
    waits = list(si.on_wait) if si is not None else []
    if waits:
        assert len(waits) <= _N_SPILL_NOPS
        for i, w in enumerate(waits):
            nops[i].ins.sync_info = bass_rust.SyncInfo(on_wait=[w], on_update=[])
        drain_inst.ins.sync_info = bass_rust.SyncInfo(
            on_wait=[], on_update=list(si.on_update))
    self.nc.all_engine_barrier()
    popped = self.nc._tile_sem_poison_stack.pop()
    assert popped is self._sem_poison
    self.nc.clear_and_free_semaphores(list(self.sems.allocated().values()))


tile.TileContext._drain_and_barrier = _patched_drain_and_barrier

# Same walrus limitation for regular scheduled instructions: split any
# multi-wait instruction into single-wait same-engine NOPs + the instruction
# keeping its last wait (sequential waits on one engine are equivalent).
_orig_lower_ordered = tile.TileContext._lower_ordered_insts


def _split_multiwait_lower(self, ordered):
    for bb, insts in ordered.items():
        out = []
        for inst in insts:
            si = inst.sync_info
            waits = list(si.on_wait) if si is not None else []
            if len(waits) > 1:
                for w in waits[:-1]:
                    out.append(mybir.InstNoOp(
                        name=self.nc.get_next_instruction_name(),
                        sync_info=mybir.SyncInfo(on_wait=[w], on_update=[]),
                        engine=inst.engine,
                        bass_nofuse=True,
                        text_hint="waitsplit",
                    ))
                inst.sync_info = mybir.SyncInfo(
                    on_wait=[waits[-1]], on_update=list(si.on_update))
            out.append(inst)
        ordered[bb] = out
    return _orig_lower_ordered(self, ordered)


tile.TileContext._lower_ordered_insts = _split_multiwait_lower


# ---------------------------------------------------------------------------
def _pack_fills(group_sizes):
    """Pack label groups (in label order) into PSUM fills of <= FILL_MAX cols.

    Returns (fills, group_fill, group_off): fills = list of (col_start, width);
    group_fill[v] = fill index of label v; group_off[v] = column offset of
    label v inside its fill. Zero-size groups get the current fill.
    """
    fills = []
    group_fill = [0] * len(group_sizes)
    group_off = [0] * len(group_sizes)
    cur_start, cur_w = 0, 0
    for v, g in enumerate(group_sizes):
        g = int(g)
        if cur_w > 0 and cur_w + g > FILL_MAX:
            fills.append((cur_start, cur_w))
            cur_start, cur_w = cur_start + cur_w, 0
        group_fill[v] = len(fills)
        group_off[v] = cur_w
        cur_w += g
    if cur_w > 0:
        fills.append((cur_start, cur_w))
    return fills, group_fill, group_off


def _build_program(row_segs, dir_layouts):
    """Build the SPMD Bass program (shared by all 8 cores).

    row_segs: list of (p_global_start, p_global_end, v) label segments over the
      RPC padded per-core rows (v = -1 for pad rows).
    dir_layouts: per direction dict with fills, group_fill, group_off,
      group_sizes (list per label).
    """
    kow = max(512, max(max(l["group_sizes"]) for l in dir_layouts) + 4)
    nc = bass.Bass("TRN2", target_bir_lowering=False, debug=False,
                   num_devices=N_CORES)
    f32, bf16 = mybir.dt.float32, mybir.dt.bfloat16
    AX = mybir.AxisListType.X
    OP = mybir.AluOpType
    ACT = mybir.ActivationFunctionType

    d_bank = [nc.dram_tensor(f"bank{d}", [2, 128, N], bf16,
                             kind="ExternalInput").ap() for d in range(2)]
    d_fT = [nc.dram_tensor(f"f{d}T", [2, 128, RPC], bf16,
                           kind="ExternalInput").ap() for d in range(2)]
    d_rm = [nc.dram_tensor(f"f{d}rm", [128, NT * C], bf16,
                           kind="ExternalInput").ap() for d in range(2)]
    d_pg = [nc.dram_tensor(f"pg{d}", [128, NT], f32,
                           kind="ExternalInput").ap() for d in range(2)]
    d_out = nc.dram_tensor("partials", [128, 4], f32, kind="ExternalOutput").ap()

    with tile.TileContext(nc) as tc:
        import contextlib
        with contextlib.ExitStack() as ctx:
            singles = ctx.enter_context(tc.tile_pool(name="singles", bufs=1))
            psum = ctx.enter_context(tc.tile_pool(name="psum", bufs=4, space="PSUM"))
            stats = ctx.enter_context(tc.tile_pool(name="stats", bufs=14))
            scratch = ctx.enter_context(tc.tile_pool(name="scratch", bufs=10))

            # ---- resident inputs ----
            bank = [[singles.tile([128, N], bf16, tag=f"bank{d}k{k}", name=f"bank{d}k{k}")
                     for k in range(2)] for d in range(2)]
            fT = [[singles.tile([128, RPC], bf16, tag=f"fT{d}k{k}", name=f"fT{d}k{k}")
                   for k in range(2)] for d in range(2)]
            rm = [singles.tile([128, NT * C], bf16, tag=f"rm{d}", name=f"rm{d}") for d in range(2)]
            pg = [singles.tile([128, NT], f32, tag=f"pg{d}", name=f"pg{d}") for d in range(2)]
            # Load order matters for the pipeline head: direction 0's first
            # fill needs fT0 + the first bank0 column chunk, so those go out
            # first; rm/pg unblock the (cheap) pos/pm prework.
            BCH = 1000
            for k in range(2):
                nc.sync.dma_start(out=fT[0][k], in_=d_fT[0][k])
                nc.sync.dma_start(out=bank[0][k][:, 0:BCH], in_=d_bank[0][k][:, 0:BCH])
            H = NT * C // 2
            for d in range(2):
                nc.sync.dma_start(out=rm[d][:, :H], in_=d_rm[d][:, :H])
                nc.sync.dma_start(out=pg[d], in_=d_pg[d])
            for k in range(2):
                nc.sync.dma_start(out=fT[1][k], in_=d_fT[1][k])
                nc.sync.dma_start(out=bank[1][k][:, 0:BCH], in_=d_bank[1][k][:, 0:BCH])
            for d in range(2):
                nc.sync.dma_start(out=rm[d][:, H:], in_=d_rm[d][:, H:])
            for cst in range(BCH, N, BCH):
                w = min(BCH, N - cst)
                for d in range(2):
                    for k in range(2):
                        nc.sync.dma_start(out=bank[d][k][:, cst:cst + w],
                                          in_=d_bank[d][k][:, cst:cst + w])

            # ---- per-direction row stats ----
            pos = [singles.tile([128, NT], f32, tag=f"pos{d}", name=f"pos{d}") for d in range(2)]
            pm = [singles.tile([128, NT], f32, tag=f"pm{d}", name=f"pm{d}") for d in range(2)]
            mcol = [singles.tile([128, NT], f32, tag=f"mcol{d}", name=f"mcol{d}") for d in range(2)]
            scol = [singles.tile([128, NT], f32, tag=f"scol{d}", name=f"scol{d}") for d in range(2)]
            loss = [singles.tile([128, NT], f32, tag=f"loss{d}", name=f"loss{d}") for d in range(2)]

            # pos[:, t] = sum_c f1[row, c] * f2[row, c] * (1/TEMP); same for
            # both directions (stop_gradient only affects backward). Computed
            # lazily inside chain(0, t) so the DVE work lands in pipeline gaps.
            negpos = singles.tile([128, NT], f32, tag="negpos", name="negpos")

            def emit_pos(t):
                prod = stats.tile([128, C], bf16, tag="prod", name="prod")
                a = rm[0][:, t * C:(t + 1) * C]
                b = rm[1][:, t * C:(t + 1) * C]
                nc.vector.tensor_tensor(out=prod, in0=a, in1=b, op=OP.mult)
                psr = stats.tile([128, 1], f32, tag="psr", name="psr")
                nc.vector.reduce_sum(out=psr, in_=prod, axis=AX)
                nc.scalar.activation(out=pos[0][:, t:t + 1], in_=psr,
                                     func=ACT.Copy, scale=1.0 / TEMP)
                nc.scalar.activation(out=negpos[:, t:t + 1], in_=psr,
                                     func=ACT.Copy, scale=-1.0 / TEMP)
                nc.gpsimd.tensor_copy(out=pos[1][:, t:t + 1],
                                      in_=pos[0][:, t:t + 1])

            # pm1 = (pg2 > thr) & (pg1 < pg2); pm2 = (pg1 > thr) & (pg2 < pg1)
            for d in range(2):
                o = 1 - d
                g = stats.tile([128, NT], f32, tag="pmg", name="pmg")
                l = stats.tile([128, NT], f32, tag="pml", name="pml")
                nc.vector.tensor_single_scalar(out=g, in_=pg[o], scalar=POS_THRESH,
                                               op=OP.is_gt)
                nc.vector.tensor_tensor(out=l, in0=pg[d], in1=pg[o], op=OP.is_lt)
                nc.vector.tensor_tensor(out=pm[d], in0=g, in1=l, op=OP.mult)

            # label-run selector: sel[p, t*RMAX + j] = 1 iff row p of tile t
            # belongs to run j (host-precomputed; identical across cores).
            d_sel = nc.dram_tensor("selind", [128, NT * RMAX], f32,
                                   kind="ExternalInput").ap()
            sel = singles.tile([128, NT * RMAX], f32, tag="selind", name="selind")
            nc.sync.dma_start(out=sel, in_=d_sel)

            # ---- main loop ----
            # The per-fill chain is PE matmul -> DVE max -> ACT exp; a single
            # chain round-trips through the 2 PSUM slots at (PE+DVE+ACT)/2 per
            # fill. Interleaving the two directions' chains (independent work)
            # keeps every engine busy: steady state ~= max(engine) per fill.
            # Killed-range sums alternate between ACT (re-exp from PSUM) and
            # DVE (reduce of the bf16 exp output) to balance the two engines.
            kill_parity = [0]

            def chain(d, t):
                lay = dir_layouts[d]
                fills = lay["fills"]
                nf = len(fills)
                runs = []
                for (s0, s1, v) in row_segs:
                    p0, p1 = max(s0, t * 128), min(s1, (t + 1) * 128)
                    if p0 < p1 and v >= 0:
                        runs.append((p0 - t * 128, p1 - t * 128, v))
                assert len(runs) <= RMAX

                negm = stats.tile([128, nf], f32, tag="negm", name="negm")
                sparts = stats.tile([128, nf], f32, tag="sparts", name="sparts")
                kaccs = stats.tile([128, RMAX], f32, tag="kaccs", name="kaccs")
                nc.gpsimd.memset(kaccs, 0.0)
                if d == 0:
                    emit_pos(t)
                lhs = [fT[d][k][:, t * 128:(t + 1) * 128] for k in range(2)]

                for fi, (cst, w) in enumerate(fills):
                    ps = psum.tile([128, FILL_MAX], f32, tag="ps", name="ps")
                    for k in range(2):
                        off = 0
                        while off < w:
                            cw = min(MM_CHUNK, w - off)
                            nc.tensor.matmul(
                                ps[:, off:off + cw], lhs[k],
                                bank[d][k][:, cst + off:cst + off + cw],
                                start=(k == 0), stop=(k == 1))
                            off += cw
                    # per-row max of this fill (negated for the exp bias)
                    nc.vector.reduce_max(out=negm[:, fi:fi + 1], in_=ps[:, :w],
                                         axis=AX, negate=True)
                    # exp(s - m_f) with row-sum accumulation
                    eo = scratch.tile([128, FILL_MAX], bf16, tag="eo", name="eo")
                    nc.scalar.activation(
                        out=eo[:, :w], in_=ps[:, :w], func=ACT.Exp,
                        bias=negm[:, fi:fi + 1], scale=1.0,
                        accum_out=sparts[:, fi:fi + 1])
                    # killed (label-equal) ranges in this fill; full-128-row
                    # group sums (partition slices must be quadrant-aligned),
                    # row-selected later via the selector matrix.
                    for j, (p0, p1, v) in enumerate(runs):
                        if lay["group_fill"][v] != fi or lay["group_sizes"][v] == 0:
                            continue
                        gw = lay["group_sizes"][v]
                        go = lay["group_off"][v]
                        if kill_parity[0] % 3 < 1:
                            ko = scratch.tile([128, kow], bf16, tag="ko", name="ko")
                            nc.scalar.activation(
                                out=ko[:, :gw], in_=ps[:, go:go + gw],
                                func=ACT.Exp, bias=negm[:, fi:fi + 1],
                                scale=1.0, accum_out=kaccs[:, j:j + 1])
                        else:
                            nc.vector.reduce_sum(out=kaccs[:, j:j + 1],
                                                 in_=eo[:, go:go + gw], axis=AX)
                        kill_parity[0] += 1
                    yield

                # flash combine in the negated domain: nm1 = -max(max_f m_f, pos)
                nmf = stats.tile([128, 1], f32, tag="nmf", name="nmf")
                nc.vector.tensor_reduce(out=nmf, in_=negm, axis=AX, op=OP.min)
                nm1 = stats.tile([128, 1], f32, tag="nm1", name="nm1")
                nc.vector.tensor_tensor(out=nm1, in0=nmf,
                                        in1=negpos[:, t:t + 1], op=OP.min)
                nc.gpsimd.tensor_copy(out=mcol[d][:, t:t + 1], in_=nm1)
                yield
                # edel_f = exp(m_f - m) = exp(-negm_f + nm1)
                edel = stats.tile([128, nf], f32, tag="edel", name="edel")
                nc.scalar.activation(out=edel, in_=negm, func=ACT.Exp,
                                     bias=nm1, scale=-1.0)
                # S_all = sum_f sparts_f * edel_f
                sprod = stats.tile([128, nf], f32, tag="sprod", name="sprod")
                nc.vector.tensor_tensor(out=sprod, in0=sparts, in1=edel,
                                        op=OP.mult)
                sall = stats.tile([128, 1], f32, tag="sall", name="sall")
                nc.vector.reduce_sum(out=sall, in_=sprod, axis=AX)
                # killed total: sum_j kaccs_j * edel[fill(v_j)] * sel_j
                edelg = stats.tile([128, RMAX], f32, tag="edelg", name="edelg")
                nc.gpsimd.memset(edelg, 0.0)
                for j, (p0, p1, v) in enumerate(runs):
                    fv = lay["group_fill"][v]
                    nc.gpsimd.tensor_copy(out=edelg[:, j:j + 1],
                                          in_=edel[:, fv:fv + 1])
                yield
                kprod = stats.tile([128, RMAX], f32, tag="kprod", name="kprod")
                nc.vector.tensor_tensor(out=kprod, in0=kaccs, in1=edelg,
                                        op=OP.mult)
                kprod2 = stats.tile([128, RMAX], f32, tag="kprod2", name="kprod2")
                nc.vector.tensor_tensor(
                    out=kprod2, in0=kprod,
                    in1=sel[:, t * RMAX:t * RMAX + RMAX], op=OP.mult)
                ks = stats.tile([128, 1], f32, tag="ks", name="ks")
                nc.vector.reduce_sum(out=ks, in_=kprod2, axis=AX)
                nc.vector.tensor_tensor(out=scol[d][:, t:t + 1], in0=sall,
                                        in1=ks, op=OP.subtract)
                yield

            from collections import deque
            pending = deque((d, t) for d in range(2) for t in range(NT))
            alive = []
            while pending and len(alive) < 10:
                d0_, t0_ = pending.popleft()
                alive.append(chain(d0_, t0_))
            while alive:
                for g in list(alive):
                    try:
                        next(g)
                    except StopIteration:
                        alive.remove(g)
                        if pending:
                            d0_, t0_ = pending.popleft()
                            alive.append(chain(d0_, t0_))

            # ---- final math per direction, batched over row tiles ----
            outt = singles.tile([128, 4], f32, tag="outt", name="outt")
            for d in range(2):
                # mcol holds -m, so pos - m = pos + mcol
                pd = stats.tile([128, NT], f32, tag="pd", name="pd")
                nc.vector.tensor_tensor(out=pd, in0=pos[d], in1=mcol[d],
                                        op=OP.add)
                num = stats.tile([128, NT], f32, tag="num", name="num")
                nc.scalar.activation(out=num, in_=pd, func=ACT.Exp)
                stot = stats.tile([128, NT], f32, tag="stot", name="stot")
                nc.vector.tensor_tensor(out=stot, in0=scol[d], in1=num, op=OP.add)
                den = stats.tile([128, NT], f32, tag="den", name="den")
                nc.vector.tensor_single_scalar(out=den, in_=stot, scalar=EPS,
                                               op=OP.add)
                rec = stats.tile([128, NT], f32, tag="rec", name="rec")
                nc.vector.reciprocal(out=rec, in_=den)
                lg = stats.tile([128, NT], f32, tag="lg", name="lg")
                nc.vector.tensor_tensor(out=lg, in0=num, in1=rec, op=OP.mult)
                lga = stats.tile([128, NT], f32, tag="lga", name="lga")
                nc.vector.tensor_single_scalar(out=lga, in_=lg, scalar=EPS, op=OP.add)
                ll = stats.tile([128, NT], f32, tag="ll", name="ll")
                nc.scalar.activation(out=ll, in_=lga, func=ACT.Ln)
                nc.vector.tensor_tensor(out=loss[d], in0=ll, in1=pm[d], op=OP.mult)
                nc.vector.reduce_sum(out=outt[:, 2 * d:2 * d + 1], in_=loss[d],
                                     axis=AX)
                nc.vector.reduce_sum(out=outt[:, 2 * d + 1:2 * d + 2], in_=pm[d],
                                     axis=AX)
            nc.sync.dma_start(out=d_out, in_=outt)

    return nc


# ---------------------------------------------------------------------------
def kernel(output_feat1, output_feat2, pseudo_label1, pseudo_label2,
           pseudo_logits1, pseudo_logits2, output_ul1, output_ul2,
           selected_idx1, selected_idx2):
    f1 = np.ascontiguousarray(np.asarray(output_feat1, dtype=np.float32))
    f2 = np.ascontiguousarray(np.asarray(output_feat2, dtype=np.float32))
    pl1 = np.asarray(pseudo_label1).astype(np.int64)
    pl2 = np.asarray(pseudo_label2).astype(np.int64)
    pg1 = np.asarray(pseudo_logits1, dtype=np.float32)
    pg2 = np.asarray(pseudo_logits2, dtype=np.float32)
    ul1 = np.asarray(output_ul1, dtype=np.float32)
    ul2 = np.asarray(output_ul2, dtype=np.float32)
    idx1 = np.asarray(selected_idx1).astype(np.int64)
    idx2 = np.asarray(selected_idx2).astype(np.int64)

    b, c, h, w = ul1.shape
    ul1f = ul1.transpose(0, 2, 3, 1).reshape(-1, c)
    ul2f = ul2.transpose(0, 2, 3, 1).reshape(-1, c)
    bank_vals = np.concatenate([ul1f[idx1], ul2f[idx2]], axis=0)   # [N, C]
    ml = np.concatenate([pl1[idx1], pl2[idx2]], axis=0)            # [N]

    # --- column layout per direction (transposed-bug mask: col k label pl_d[k])
    dir_layouts, banks = [], []
    for pl in (pl1, pl2):
        order = np.argsort(pl, kind="stable")
        sizes = np.bincount(pl, minlength=NLAB).tolist()
        fills, gfill, goff = _pack_fills(sizes)
        dir_layouts.append(dict(fills=fills, group_fill=gfill, group_off=goff,
                                group_sizes=sizes))
        bT = np.ascontiguousarray(bank_vals[order].T.astype(BF16))  # [C, N]
        banks.append(bT.reshape(2, 128, N))

    # --- row layout: label-sorted with fixed per-core quotas
    nv = np.bincount(ml, minlength=NLAB)
    qv = (nv + N_CORES - 1) // N_CORES
    assert qv.sum() <= RPC
    row_segs = []
    p = 0
    for v in range(NLAB):
        if qv[v] > 0:
            row_segs.append((p, p + int(qv[v]), v))
            p += int(qv[v])
    if p < RPC:
        row_segs.append((p, RPC, -1))

    global RMAX
    need = max(sum(1 for (s0, s1, v) in row_segs
                   if v >= 0 and max(s0, t * 128) < min(s1, (t + 1) * 128))
               for t in range(NT))
    RMAX = max(6, need)

    rows_sorted = np.argsort(ml, kind="stable")
    starts = np.concatenate([[0], np.cumsum(nv)])
    perms = np.full((N_CORES, RPC), -1, dtype=np.int64)
    for v in range(NLAB):
        seg = next(s for s in row_segs if s[2] == v)
        rows_v = rows_sorted[starts[v]:starts[v + 1]]
        for core in range(N_CORES):
            chunk = rows_v[core * qv[v]:(core + 1) * qv[v]]
            perms[core, seg[0]:seg[0] + len(chunk)] = chunk

    # run selector: sel[p, t*RMAX + j] = 1 iff padded row t*128+p is in run j
    selind = np.zeros((128, NT * RMAX), dtype=np.float32)
    for t in range(NT):
        j = 0
        for (s0, s1, v) in row_segs:
            p0, p1 = max(s0, t * 128), min(s1, (t + 1) * 128)
            if p0 < p1 and v >= 0:
                selind[p0 - t * 128:p1 - t * 128, t * RMAX + j] = 1.0
                j += 1
        assert j <= RMAX

    # --- per-core input maps
    def gather_rows(x, perm):
        out = np.zeros((RPC,) + x.shape[1:], dtype=x.dtype)
        msk = perm >= 0
        out[msk] = x[perm[msk]]
        return out

    in_maps = []
    for core in range(N_CORES):
        perm = perms[core]
        fc = [gather_rows(f1, perm), gather_rows(f2, perm)]
        pgc = [gather_rows(pg1, perm), gather_rows(pg2, perm)]
        m = {"selind": selind}
        for d in range(2):
            m[f"bank{d}"] = banks[d]
            fT = np.ascontiguousarray((fc[d].T * (1.0 / TEMP)).astype(BF16))
            m[f"f{d}T"] = fT.reshape(2, 128, RPC)
            m[f"f{d}rm"] = np.ascontiguousarray(
                fc[d].reshape(NT, 128, C).transpose(1, 0, 2).reshape(128, NT * C)
                .astype(BF16))
            m[f"pg{d}"] = np.ascontiguousarray(pgc[d].reshape(NT, 128).T)
        in_maps.append(m)

    nc = _build_program(row_segs, dir_layouts)
    res = run_bass_kernel_spmd(nc, in_maps, list(range(N_CORES)))
    global LAST_RESULTS
    LAST_RESULTS = res

    tot = np.zeros(4, dtype=np.float64)
    for core in range(N_CORES):
        tot += res.results[core]["partials"].astype(np.float64).sum(axis=0)
    loss1 = -tot[0] / (tot[1] + 1e-12)
    loss2 = -tot[2] / (tot[3] + 1e-12)
    return np.float32(loss1 + loss2)



# revision 3
# speedup vs baseline: 6.2921x; 6.2921x over previous
"""Trainium2 Bass kernel for nn_DirectionalContrastiveLoss (8-core SPMD).

Algorithmic structure
---------------------
The reference loss is  mean over masked rows of  -log(lg + 1e-8)  with
lg = exp(pos - M) / (S_masked + 1e-8),  M = row max over [pos, scores].
Because of the 1e-8 clamp inside the log, any row whose pos is more than
~43 below its row max contributes exactly -log(1e-8): lg <= e^{pos-M}*1e8
is then < 2e-11 and shifts the log by < 1e-2 * 1e-8. With randn features
the scores have std ~160 and row maxes ~600, so only a handful of rows in
the whole problem deviate from the clamp.

The device therefore only needs (a) the score matmul for the masked rows
(25.6% of rows appear in the loss at all) and (b) a per-row approximate
max. A subsampled max can only UNDER-estimate, which only widens the
host-side selection - never corrupts it. The few selected rows (plus any
rows beyond the device tile capacity) get an exact float64 masked softmax
on the host; every other masked row contributes the clamp constant.

Device kernel per core: fp8 DoubleRow matmuls (both 128-deep k-tiles of
the K=256 contraction in one pass) of [128-row tile] x [8000-col bank,
replicated, order-free] accumulating fp32 in PSUM, then a stride-4
reduce_max per 1024-col fill. No exp / softmax machinery on device.
"""
import numpy as np
import ml_dtypes

import bass_rust
import concourse.bass as bass
import concourse.tile as tile
from concourse import mybir
from concourse.bass_utils import run_bass_kernel_spmd
from concourse.vector_clock import ScopedClock

F8 = ml_dtypes.float8_e4m3   # TRN fp8e4: max normal +-240
N_CORES = 8
TEMP = 0.1
POS_THRESH = 0.7
EPS = 1e-8
N = 8000          # anchors (== memory slots)
C = 256           # feature channels
FILL = 1024       # PSUM fill width (2 banks of fp32)
SUB = 4           # reduce_max column subsample stride
MARGIN = 110.0    # selection threshold slack (clamp 43 + fp8 max-error tail)
HOST_CAP = 64     # max rows/direction computed on host due to capacity

LAST_RESULTS = None  # BassKernelResults of the most recent kernel() call

# ---------------------------------------------------------------------------
# walrus in this toolchain rejects >1 sync wait per instruction; spread the
# TileContext tail-drain waits over single-wait sync NOPs.
_N_SPILL_NOPS = 64


def _patched_drain_and_barrier(self, tick_clock, wait_clock):
    nops = [self.nc.sync.nop(nofuse=True, hint=f"drainwait{i}")
            for i in range(_N_SPILL_NOPS)]
    drain_inst = self.nc.sync.drain()
    wait_clock.add_sem_waits(drain_inst.ins,
                             ScopedClock({None: tick_clock.global_clock}))
    si = drain_inst.ins.sync_info
    waits = list(si.on_wait) if si is not None else []
    if waits:
        assert len(waits) <= _N_SPILL_NOPS
        for i, w in enumerate(waits):
            nops[i].ins.sync_info = bass_rust.SyncInfo(on_wait=[w], on_update=[])
        drain_inst.ins.sync_info = bass_rust.SyncInfo(
            on_wait=[], on_update=list(si.on_update))
    self.nc.all_engine_barrier()
    popped = self.nc._tile_sem_poison_stack.pop()
    assert popped is self._sem_poison
    self.nc.clear_and_free_semaphores(list(self.sems.allocated().values()))


tile.TileContext._drain_and_barrier = _patched_drain_and_barrier

# Same walrus limitation for regular scheduled instructions: split any
# multi-wait instruction into single-wait same-engine NOPs + the instruction
# keeping its last wait (sequential waits on one engine are equivalent).
_orig_lower_ordered = tile.TileContext._lower_ordered_insts


def _split_multiwait_lower(self, ordered):
    for bb, insts in ordered.items():
        out = []
        for inst in insts:
            si = inst.sync_info
            waits = list(si.on_wait) if si is not None else []
            if len(waits) > 1:
                for w in waits[:-1]:
                    out.append(mybir.InstNoOp(
                        name=self.nc.get_next_instruction_name(),
                        sync_info=mybir.SyncInfo(on_wait=[w], on_update=[]),
                        engine=inst.engine,
                        bass_nofuse=True,
                        text_hint="waitsplit",
                    ))
                inst.sync_info = mybir.SyncInfo(
                    on_wait=[waits[-1]], on_update=list(si.on_update))
            out.append(inst)
        ordered[bb] = out
    return _orig_lower_ordered(self, ordered)


tile.TileContext._lower_ordered_insts = _split_multiwait_lower


# ---------------------------------------------------------------------------
def _fills():
    out = []
    c = 0
    while c < N:
        out.append((c, min(FILL, N - c)))
        c += FILL
    return out


def _build_program(ntot):
    """SPMD program: ntot 128-row tiles x full bank matmul + subsampled max."""
    nc = bass.Bass("TRN2", target_bir_lowering=False, debug=False,
                   num_devices=N_CORES)
    f8, f32 = mybir.dt.float8e4, mybir.dt.float32
    AX = mybir.AxisListType.X
    DR = mybir.MatmulPerfMode.DoubleRow
    fills = _fills()
    nf = len(fills)

    d_bank = nc.dram_tensor("bank", [128, 2, N], f8, kind="ExternalInput").ap()
    d_fT = nc.dram_tensor("fT", [128, 2, ntot * 128], f8,
                          kind="ExternalInput").ap()
    d_out = nc.dram_tensor("negm", [128, ntot * nf], f32,
                           kind="ExternalOutput").ap()

    with tile.TileContext(nc) as tc:
        import contextlib
        with contextlib.ExitStack() as ctx:
            singles = ctx.enter_context(tc.tile_pool(name="singles", bufs=1))
            psum = ctx.enter_context(tc.tile_pool(name="psum", bufs=4,
                                                  space="PSUM"))
            bank = singles.tile([128, 2, N], f8, tag="bank", name="bank")
            fT = singles.tile([128, 2, ntot * 128], f8, tag="fT", name="fT")
            outm = singles.tile([128, ntot * nf], f32, tag="outm", name="outm")

            nc.sync.dma_start(out=fT, in_=d_fT)
            CH = 2048  # bank DMA chunk: 2 fills per chunk, streams ahead of PE
            for c0 in range(0, N, CH):
                w = min(CH, N - c0)
                nc.sync.dma_start(out=bank[:, :, c0:c0 + w],
                                  in_=d_bank[:, :, c0:c0 + w])

            for s in range(ntot):
                lhs = fT[:, :, s * 128:(s + 1) * 128]
                for fi, (c0, w) in enumerate(fills):
                    ps = psum.tile([128, FILL], f32, tag="ps", name="ps")
                    for h0 in range(0, w, 512):
                        hw = min(512, w - h0)
                        nc.tensor.matmul(
                            ps[:, h0:h0 + hw], lhs,
                            bank[:, :, c0 + h0:c0 + h0 + hw],
                            start=True, stop=True, perf_mode=DR)
                    nc.vector.reduce_max(
                        out=outm[:, s * nf + fi:s * nf + fi + 1],
                        in_=ps[:, 0:w:SUB], axis=AX)
            nc.sync.dma_start(out=d_out, in_=outm)

    return nc


# ---------------------------------------------------------------------------
def kernel(output_feat1, output_feat2, pseudo_label1, pseudo_label2,
           pseudo_logits1, pseudo_logits2, output_ul1, output_ul2,
           selected_idx1, selected_idx2):
    f1 = np.ascontiguousarray(np.asarray(output_feat1, dtype=np.float32))
    f2 = np.ascontiguousarray(np.asarray(output_feat2, dtype=np.float32))
    pl1 = np.asarray(pseudo_label1).astype(np.int64)
    pl2 = np.asarray(pseudo_label2).astype(np.int64)
    pg1 = np.asarray(pseudo_logits1, dtype=np.float32)
    pg2 = np.asarray(pseudo_logits2, dtype=np.float32)
    ul1 = np.asarray(output_ul1, dtype=np.float32)
    ul2 = np.asarray(output_ul2, dtype=np.float32)
    idx1 = np.asarray(selected_idx1).astype(np.int64)
    idx2 = np.asarray(selected_idx2).astype(np.int64)

    b, c, h, w = ul1.shape
    ul1f = ul1.transpose(0, 2, 3, 1).reshape(-1, c)
    ul2f = ul2.transpose(0, 2, 3, 1).reshape(-1, c)
    memory = np.concatenate([ul1f[idx1], ul2f[idx2]], axis=0)     # [N, C]
    ml = np.concatenate([pl1[idx1], pl2[idx2]], axis=0)           # [N]

    pm = [((pg2 > POS_THRESH) & (pg1 < pg2)),
          ((pg1 > POS_THRESH) & (pg2 < pg1))]
    anchors = [f1, f2]
    alabels = [pl1, pl2]
    rows = [np.nonzero(m)[0] for m in pm]
    counts = [len(r) for r in rows]

    # device capacity: nt[d] 128-row tiles per core per direction; at most
    # HOST_CAP overflow rows per direction fall back to exact host compute
    nt = [max(1, -(-(max(cnt - HOST_CAP, 1)) // (128 * N_CORES)))
          for cnt in counts]
    cap = [t * 128 * N_CORES for t in nt]
    dev_rows = [r[:cp] for r, cp in zip(rows, cap)]
    host_rows = [r[cp:] for r, cp in zip(rows, cap)]
    ntot = nt[0] + nt[1]
    nf = len(_fills())

    # pos (score of the positive pair) for all masked rows, float64 on host
    pos_all = (f1.astype(np.float64) * f2.astype(np.float64)).sum(1) / TEMP

    # --- per-core fp8 stationary operand: [128, 2, ntot*128]
    # element (p, i, r) = channel i*128+p of local row r; dir-1 tiles first
    per_core = [t * 128 for t in nt]
    in_maps = []
    bankT = memory.T.astype(np.float32)                           # [C, N]
    bank_dev = np.clip(bankT, -240, 240).reshape(2, 128, N).transpose(1, 0, 2)
    bank_dev = np.ascontiguousarray(bank_dev.astype(F8))
    for core in range(N_CORES):
        cols = np.zeros((C, ntot * 128), dtype=np.float32)
        off = 0
        for d in range(2):
            sl = dev_rows[d][core * per_core[d]:(core + 1) * per_core[d]]
            if len(sl):
                cols[:, off:off + len(sl)] = anchors[d][sl].T / TEMP
            off += per_core[d]
        fTc = np.clip(cols, -240, 240).reshape(2, 128, ntot * 128)
        fTc = np.ascontiguousarray(fTc.transpose(1, 0, 2).astype(F8))
        in_maps.append({"bank": bank_dev, "fT": fTc})

    nc = _build_program(ntot)
    res = run_bass_kernel_spmd(nc, in_maps, list(range(N_CORES)))
    global LAST_RESULTS
    LAST_RESULTS = res

    # --- decode per-row approximate maxes
    mhat = [np.full(len(dev_rows[d]), -np.inf) for d in range(2)]
    for core in range(N_CORES):
        o = res.results[core]["negm"].astype(np.float64)          # [128, ntot*nf]
        for d in range(2):
            base_slot = 0 if d == 0 else nt[0]
            for t in range(nt[d]):
                g0 = core * per_core[d] + t * 128
                take = min(128, len(dev_rows[d]) - g0)
                if take <= 0:
                    continue
                sl = slice((base_slot + t) * nf, (base_slot + t + 1) * nf)
                mhat[d][g0:g0 + take] = o[:take, sl].max(axis=1)

    # --- host: exact contributions for selected + overflow rows, clamp rest
    CLAMP = -np.log(np.float64(EPS))
    mem64 = memory.astype(np.float64)
    total = np.float64(0)
    for d in range(2):
        sel = dev_rows[d][pos_all[dev_rows[d]] > mhat[d] - MARGIN]
        exact = np.concatenate([sel, host_rows[d]]).astype(np.int64)
        contrib = np.float64(0)
        if len(exact):
            A = anchors[d][exact].astype(np.float64)
            S = A @ mem64.T / TEMP                                # [k, N]
            pos_e = pos_all[exact]
            M = np.maximum(S.max(axis=1), pos_e)
            keep = (alabels[d][None, :] != ml[exact][:, None])
            Ssum = (np.exp(S - M[:, None]) * keep).sum(axis=1) \
                + np.exp(pos_e - M)
            lg = np.exp(pos_e - M) / (Ssum + EPS)
            contrib = (-np.log(lg + EPS)).sum()
        loss_d = (contrib + (counts[d] - len(exact)) * CLAMP) \
            / (counts[d] + 1e-12)
        total += loss_d
    return np.float32(total)
